# revision 23
# baseline (speedup 1.0000x reference)
"""Trainium2 Bass kernel for JacobianRegulariser2D.

reference math (f32, H=W=4096):
  dy = central diff along H, dx = central diff along W (3-tap [0.5,0,-0.5], zero pad)
  crop [2:-2, 2:-2] -> 4092x4092
  det = (dy0+1)(dx1+1) - dx0*dy1;  out = mean(relu(-det)^2)

With A = ux[i-1]-ux[i+1] (rows), B likewise for uy, C = ux[j-1]-ux[j+1]
(cols), D likewise for uy:  relu(-det)^2 = (1/16) max(CB - (A+2)(D+2), 0)^2.

Negated dataflow: a2n = -(A+2), bn = -B, c' = -C, d2 = D+2,
s = c'*bn + a2n*d2; result rows sum via Square(mask*relu(s)) with accum.

Sharding: H split 8 ways; each core runs 4 row-tiles of 128 rows. Rows
whose 3-tap row stencil crosses a tile boundary (2 per tile, 64 globally)
are masked out on device and computed on the host in f64 (~1.5% of rows,
vectorized numpy) — this removes all halo machinery from the device. Per
tile: PE row stencils (single stationary banded weight, 4 FD-512 matmuls
per 1024-col chunk) into PSUM; ACT copies fan A/B out negated (scale=-1,
bias=-2 on the A half); col diffs and products on DVE at FD=4092 in bf16
(the host ships uy+2 as a third stream so D+2 is a single tensor_sub);
ACT Square(scale=mask) with accum_out forms per-tile partials, with the
last tile drained in quarters. Host reduces the [128, 8] partials plus
the host-side boundary-row term.
"""

import sys

import numpy as np

sys.path.insert(0, "/opt/trn_rl_repo")

import concourse.bass as bass  # noqa: E402
import concourse.tile as tile  # noqa: E402
from concourse import bacc, mybir  # noqa: E402
from concourse.bass_utils import run_bass_kernel_spmd  # noqa: E402

P = 128
H = 4096
W = 4096
N_CORES = 8
N_TILES = 4
ROWS = 512
OUT_COLS = 4092
NC = 4

F32 = mybir.dt.float32
BF16 = mybir.dt.bfloat16
_BF16_NP = mybir.dt.np(BF16)

Copy = mybir.ActivationFunctionType.Copy
Square = mybir.ActivationFunctionType.Square
Alu = mybir.AluOpType


def _r0(k):
    """Strip origin: out row of core k tile t partition p is _r0(k)+128t+1+p."""
    return 1 + 512 * k if k < N_CORES - 1 else H - 514


def _stencil_weights():
    """lhsT [128,128]: out[i] = in[i-1] - in[i+1]."""
    w = np.zeros((P, P), dtype=np.float32)
    idx = np.arange(P - 1)
    w[idx, idx + 1] = 1.0
    w[idx + 1, idx] = -1.0
    return w.astype(_BF16_NP)


def _build_program():
    nc = bacc.Bacc("TRN2", target_bir_lowering=False)

    ux = nc.dram_tensor("ux", [ROWS, W], BF16, kind="ExternalInput")
    uy = nc.dram_tensor("uy", [ROWS, W], BF16, kind="ExternalInput")
    uyp2 = nc.dram_tensor("uyp2", [ROWS, W], BF16, kind="ExternalInput")
    wst = nc.dram_tensor("wst", [P, P], BF16, kind="ExternalInput")
    maskd = nc.dram_tensor("mask", [P, N_TILES], F32, kind="ExternalInput")
    outd = nc.dram_tensor("out", [P, 2 * N_TILES], F32, kind="ExternalOutput")

    V = OUT_COLS

    with tile.TileContext(nc) as tc:
        with (
            tc.tile_pool(name="const", bufs=1) as const_pool,
            tc.tile_pool(name="inp", bufs=2) as inp_pool,
            tc.tile_pool(name="work", bufs=2) as work_pool,
            tc.tile_pool(name="acc", bufs=1) as acc_pool,
            tc.tile_pool(name="psum", bufs=2, space="PSUM") as psum_pool,
        ):
            wst_sb = const_pool.tile([P, P], BF16)
            nc.sync.dma_start(out=wst_sb, in_=wst[:, :])
            mask_sb = const_pool.tile([P, N_TILES], F32)
            nc.sync.dma_start(out=mask_sb, in_=maskd[:, :])
            racc = acc_pool.tile([P, 2 * N_TILES], F32)
            nc.vector.memset(racc[:, :], 0.0)

            def racc_col(t_, c0):
                if t_ < N_TILES - 1:
                    return t_
                return 4 + c0 // 1023  # last tile: one col per quarter

            pending_a = []  # tiles awaiting m1n/q/s
            pending_b = []  # tiles awaiting rs + reduce

            def flush_a(parts=1):
                a2n, bn, cp, d2, t_ = pending_a.pop(0)
                m1n = work_pool.tile([P, 4096], BF16, tag="m1n")
                q = work_pool.tile([P, 4096], BF16, tag="q")
                s = work_pool.tile([P, 4096], BF16, tag="s")
                w = V // parts
                for h in range(parts):
                    c0, c1 = h * w, min(V, (h + 1) * w)
                    nc.vector.tensor_mul(m1n[:, c0:c1], a2n[:, c0:c1], d2[:, c0:c1])
                    nc.vector.tensor_mul(q[:, c0:c1], cp[:, c0:c1], bn[:, c0:c1])
                    nc.vector.tensor_add(s[:, c0:c1], q[:, c0:c1], m1n[:, c0:c1])
                    pending_b.append((s, t_, c0, c1))

            def flush_b():
                s, t_, c0, c1 = pending_b.pop(0)
                col = racc_col(t_, c0)
                dump = work_pool.tile([P, 4096], BF16, tag="dump")
                rs = work_pool.tile([P, 4096], BF16, tag="rs")
                nc.vector.tensor_scalar_max(rs[:, c0:c1], s[:, c0:c1], 0.0)
                nc.scalar.activation(
                    dump[:, c0:c1], rs[:, c0:c1], Square,
                    scale=mask_sb[:, t_ : t_ + 1],
                    accum_out=racc[:, col : col + 1],
                )

            for t in range(N_TILES):
                # SBUF col v = HBM col v+1 (4B-aligned DVE shifted slices)
                ux_t = inp_pool.tile([P, W - 1], BF16, tag="ux_t")
                uy_t = inp_pool.tile([P, W - 1], BF16, tag="uy_t")
                uyp2_t = inp_pool.tile([P, W - 1], BF16, tag="uyp2_t")
                nc.sync.dma_start(out=ux_t, in_=ux[P * t : P * t + P, 1:W])
                nc.sync.dma_start(out=uy_t, in_=uy[P * t : P * t + P, 1:W])
                nc.sync.dma_start(out=uyp2_t, in_=uyp2[P * t : P * t + P, 1:W])

                # col diffs at FD=V: out col u+2 <- SBUF cols u, u+2
                cp = work_pool.tile([P, 4096], BF16, tag="cp")  # -C
                nc.vector.tensor_sub(cp[:, :V], ux_t[:, 2 : 2 + V], ux_t[:, 0:V])
                d2 = work_pool.tile([P, 4096], BF16, tag="d2")  # D + 2
                nc.vector.tensor_sub(d2[:, :V], uyp2_t[:, 0:V], uy_t[:, 2 : 2 + V])

                if pending_a:
                    flush_a()

                a2n = work_pool.tile([P, 4096], BF16, tag="a2n")
                bn = work_pool.tile([P, 4096], BF16, tag="bn")
                for ci in range(NC):
                    j0 = 1024 * ci
                    n_ci = min(1024, V - j0)
                    ab_ps = psum_pool.tile([P, 2048], F32, tag="ab")
                    for j in (0, 512):
                        fd = min(512, n_ci - j)
                        nc.tensor.matmul(
                            ab_ps[:, j : j + fd], wst_sb,
                            ux_t[:, j0 + j + 1 : j0 + j + 1 + fd],
                            start=True, stop=True,
                        )
                        nc.tensor.matmul(
                            ab_ps[:, 1024 + j : 1024 + j + fd], wst_sb,
                            uy_t[:, j0 + j + 1 : j0 + j + 1 + fd],
                            start=True, stop=True,
                        )
                    nc.scalar.activation(
                        a2n[:, j0 : j0 + n_ci], ab_ps[:, 0:n_ci], Copy,
                        scale=-1.0, bias=-2.0,
                    )
                    nc.scalar.activation(
                        bn[:, j0 : j0 + n_ci], ab_ps[:, 1024 : 1024 + n_ci], Copy,
                        scale=-1.0,
                    )
                pending_a.append((a2n, bn, cp, d2, t))
                if pending_b:
                    flush_b()

            while pending_a:
                flush_a(parts=4)
                while len(pending_b) > 1:
                    flush_b()
            while pending_b:
                flush_b()

            nc.sync.dma_start(out=outd[:, :], in_=racc)

    nc.compile()
    return nc


_NC_CACHE = None


def _get_program():
    global _NC_CACHE
    if _NC_CACHE is None:
        _NC_CACHE = _build_program()
    return _NC_CACHE


def _to_bf16(x):
    """f32 -> bf16 round-to-nearest-even, vectorized."""
    u = x.view(np.uint32)
    r = ((u >> 16) & 1) + np.uint32(0x7FFF)
    return ((u + r) >> 16).astype(np.uint16).view(_BF16_NP)


def _device_masks():
    """mask[k][p, t]: 1 where core k's (t, p) row is device-computed."""
    masks = []
    covered = np.zeros(H, dtype=bool)
    for k in range(N_CORES):
        r0 = _r0(k)
        out0 = 2 + 512 * k
        tt, pp = np.meshgrid(np.arange(N_TILES), np.arange(P), indexing="xy")
        rows = r0 + 1 + P * tt + pp
        own = (rows >= out0) & (rows < min(out0 + 512, H - 2))
        interior = (pp >= 1) & (pp <= P - 2)
        m = own & interior
        masks.append(m.astype(np.float32))
        covered[rows[m]] = True
    host_rows = np.nonzero(~covered[2 : H - 2])[0] + 2
    return masks, host_rows


_MASKS, _HOST_ROWS = _device_masks()


def _host_boundary_sum(disp):
    """f64 sum of relu(-det)^2 over the masked-out rows (full formula)."""
    g = _HOST_ROWS
    d = disp[0].astype(np.float64)  # [2, H, W]
    ux, uy = d[0], d[1]
    A = ux[g - 1, 2 : H - 2] - ux[g + 1, 2 : H - 2]
    B = uy[g - 1, 2 : H - 2] - uy[g + 1, 2 : H - 2]
    C = ux[g][:, 1 : H - 3] - ux[g][:, 3 : H - 1]
    D = uy[g][:, 1 : H - 3] - uy[g][:, 3 : H - 1]
    s = C * B - (A + 2.0) * (D + 2.0)
    return np.square(np.maximum(s, 0.0)).sum()


def _make_in_maps(displacement: np.ndarray):
    disp = np.asarray(displacement)
    if disp.dtype != np.float32:
        disp = disp.astype(np.float32)
    ux16 = _to_bf16(np.ascontiguousarray(disp[0, 0]))
    uy16 = _to_bf16(np.ascontiguousarray(disp[0, 1]))
    uyp2_16 = _to_bf16(np.ascontiguousarray(disp[0, 1] + 2.0))

    wst = _stencil_weights()

    in_maps = []
    for k in range(N_CORES):
        r0 = _r0(k)
        in_maps.append(
            {
                "ux": ux16[r0 + 1 : r0 + 513],
                "uy": uy16[r0 + 1 : r0 + 513],
                "uyp2": uyp2_16[r0 + 1 : r0 + 513],
                "wst": wst,
                "mask": np.ascontiguousarray(_MASKS[k]),
            }
        )
    return in_maps


def kernel(displacement: np.ndarray) -> np.ndarray:
    disp = np.asarray(displacement)
    in_maps = _make_in_maps(disp)
    nc = _get_program()
    res = run_bass_kernel_spmd(nc, in_maps, core_ids=list(range(N_CORES)))
    total = _host_boundary_sum(disp)
    for k in range(N_CORES):
        total += np.asarray(res.results[k]["out"], dtype=np.float64).sum()
    mean = total / (16.0 * OUT_COLS * OUT_COLS)
    return np.float32(mean)
